# revision 5
# baseline (speedup 1.0000x reference)
"""Fallback hybrid: FPS/ball-query/grouping on host (numpy, exact reference
semantics), shared MLPs + BN + max-pool data-parallel on the 8 NeuronCores
with psum for global BN stats."""

from concurrent.futures import ThreadPoolExecutor
from functools import partial

import jax
import jax.numpy as jnp
import numpy as np

EPS = 1e-5
N_CORES = 8
B_FULL = 32
N_PTS = 4096


def _fps_np(xyz, npoint):
    B, N, _ = xyz.shape
    idx = np.zeros((B, npoint), np.int32)
    dists = np.full((B, N), 1e10, np.float32)
    last = np.zeros((B,), np.int32)
    bidx = np.arange(B)
    for t in range(1, npoint):
        p = xyz[bidx, last][:, None, :]  # (B,1,3)
        d = xyz - p
        d2 = d[..., 0] * d[..., 0] + d[..., 1] * d[..., 1] + d[..., 2] * d[..., 2]
        np.minimum(dists, d2, out=dists)
        last = np.argmax(dists, -1).astype(np.int32)
        idx[:, t] = last
    return idx


def _bq_rows(xyz, new_xyz, r2, nsample, b0, b1, s0, s1):
    # exact reference semantics: first-nsample in-radius points in index
    # order, padded with the first hit (every center is its own first hit)
    diff = new_xyz[b0:b1, s0:s1, None, :] - xyz[b0:b1, None, :, :]
    d2 = (
        diff[..., 0] * diff[..., 0]
        + diff[..., 1] * diff[..., 1]
        + diff[..., 2] * diff[..., 2]
    )
    N = xyz.shape[1]
    mask = (d2 < r2).reshape(-1, N)
    counts = mask.sum(-1)
    row_ids, cols = np.nonzero(mask)
    excl = np.cumsum(counts) - counts
    pos = np.arange(len(cols)) - excl[row_ids]
    first = cols[np.minimum(excl, max(len(cols) - 1, 0))]
    ob = np.broadcast_to(first[:, None], (len(counts), nsample)).copy()
    keep = pos < nsample
    ob[row_ids[keep], pos[keep]] = cols[keep]
    return ob.reshape(b1 - b0, s1 - s0, nsample)


def _ball_query_np(xyz, new_xyz, radius, nsample):
    B, N, _ = xyz.shape
    S = new_xyz.shape[1]
    r2 = radius * radius
    out = np.empty((B, S, nsample), np.int64)
    jobs = []
    with ThreadPoolExecutor(max_workers=16) as ex:
        for b0 in range(0, B, 4):
            for s0 in range(0, S, 128):
                b1, s1 = min(b0 + 4, B), min(s0 + 128, S)
                jobs.append(
                    (b0, b1, s0, s1,
                     ex.submit(_bq_rows, xyz, new_xyz, r2, nsample, b0, b1, s0, s1))
                )
        for b0, b1, s0, s1, f in jobs:
            out[b0:b1, s0:s1] = f.result()
    return out


def _shared_mlp(x, layers, n_global):
    for lyr in layers:
        x = x @ lyr["W"] + lyr["b"]
        mean = jax.lax.psum(jnp.sum(x, axis=(0, 1, 2)), "dp") / n_global
        var = jax.lax.psum(jnp.sum((x - mean) ** 2, axis=(0, 1, 2)), "dp") / n_global
        x = lyr["gamma"] * (x - mean) * jax.lax.rsqrt(var + EPS) + lyr["beta"]
        x = jax.nn.relu(x)
    return x


@partial(jax.pmap, axis_name="dp", in_axes=(0, None))
def _mlp1(g1, params):
    return jnp.max(_shared_mlp(g1, params["sa1"], float(B_FULL * 512 * 32)), axis=2)


@partial(jax.pmap, axis_name="dp", in_axes=(0, None))
def _mlp2(g2, params):
    return jnp.max(_shared_mlp(g2, params["sa2"], float(B_FULL * 256 * 16)), axis=2)


@partial(jax.pmap, axis_name="dp", in_axes=(0, None))
def _mlp3(g3, params):
    f3 = jnp.max(_shared_mlp(g3, params["sa3"], float(B_FULL * 256)), axis=2)
    return jnp.transpose(f3, (0, 2, 1))


def _shard(a):
    return a.reshape((N_CORES, a.shape[0] // N_CORES) + a.shape[1:])


def _unshard(a):
    a = np.asarray(a)
    return a.reshape((a.shape[0] * a.shape[1],) + a.shape[2:])


def kernel(pointcloud, params):
    xyz = np.asarray(pointcloud, dtype=np.float32)[..., :3]
    params = jax.tree.map(lambda a: np.asarray(a, dtype=np.float32), params)
    B = xyz.shape[0]
    bidx = np.arange(B)[:, None]
    b2 = np.arange(B)[:, None, None]

    # ---- SA1 indices on host
    fps1 = _fps_np(xyz, 512)
    new_xyz1 = xyz[bidx, fps1]  # (B,512,3)
    idx1 = _ball_query_np(xyz, new_xyz1, 0.04, 32)
    g1 = xyz[b2, idx1] - new_xyz1[:, :, None]  # (B,512,32,3)

    f1 = _unshard(_mlp1(_shard(g1), params))  # (B,512,128)

    # ---- SA2
    fps2 = _fps_np(new_xyz1, 256)
    new_xyz2 = new_xyz1[bidx, fps2]
    idx2 = _ball_query_np(new_xyz1, new_xyz2, 0.10, 16)
    g2 = np.concatenate(
        [new_xyz1[b2, idx2] - new_xyz2[:, :, None], f1[b2, idx2]], -1
    )  # (B,256,16,131)

    f2 = _unshard(_mlp2(_shard(g2), params))  # (B,256,256)

    # ---- SA3 (group all)
    g3 = np.concatenate([new_xyz2[:, None], f2[:, None]], -1)  # (B,1,256,259)
    out = _unshard(_mlp3(_shard(g3), params))  # (B,1024,1)
    return np.asarray(out, dtype=np.float32)


# revision 6
# speedup vs baseline: 10.8565x; 10.8565x over previous
"""Hybrid v2: FPS + ball query on host; grouping gathers AND shared MLPs on
the 8 NeuronCores, with activations kept device-resident between stages to
minimize axon transfer traffic. BN stats are exact global-batch via psum."""

from concurrent.futures import ThreadPoolExecutor
from functools import partial

import jax
import jax.numpy as jnp
import numpy as np

EPS = 1e-5
N_CORES = 8
B_FULL = 32
N_PTS = 4096


def _fps_np(xyz, npoint):
    B, N, _ = xyz.shape
    idx = np.zeros((B, npoint), np.int32)
    dists = np.full((B, N), 1e10, np.float32)
    last = np.zeros((B,), np.int32)
    bidx = np.arange(B)
    for t in range(1, npoint):
        p = xyz[bidx, last][:, None, :]
        d = xyz - p
        d2 = d[..., 0] * d[..., 0] + d[..., 1] * d[..., 1] + d[..., 2] * d[..., 2]
        np.minimum(dists, d2, out=dists)
        last = np.argmax(dists, -1).astype(np.int32)
        idx[:, t] = last
    return idx


def _bq_rows(xyz, new_xyz, r2, nsample, b0, b1, s0, s1):
    diff = new_xyz[b0:b1, s0:s1, None, :] - xyz[b0:b1, None, :, :]
    d2 = (
        diff[..., 0] * diff[..., 0]
        + diff[..., 1] * diff[..., 1]
        + diff[..., 2] * diff[..., 2]
    )
    N = xyz.shape[1]
    mask = (d2 < r2).reshape(-1, N)
    counts = mask.sum(-1)
    row_ids, cols = np.nonzero(mask)
    excl = np.cumsum(counts) - counts
    pos = np.arange(len(cols)) - excl[row_ids]
    first = cols[np.minimum(excl, max(len(cols) - 1, 0))]
    ob = np.broadcast_to(first[:, None], (len(counts), nsample)).copy()
    keep = pos < nsample
    ob[row_ids[keep], pos[keep]] = cols[keep]
    return ob.reshape(b1 - b0, s1 - s0, nsample)


def _ball_query_np(xyz, new_xyz, radius, nsample):
    B, N, _ = xyz.shape
    S = new_xyz.shape[1]
    r2 = radius * radius
    out = np.empty((B, S, nsample), np.int32)
    jobs = []
    with ThreadPoolExecutor(max_workers=16) as ex:
        for b0 in range(0, B, 4):
            for s0 in range(0, S, 128):
                b1, s1 = min(b0 + 4, B), min(s0 + 128, S)
                jobs.append(
                    (b0, b1, s0, s1,
                     ex.submit(_bq_rows, xyz, new_xyz, r2, nsample, b0, b1, s0, s1))
                )
        for b0, b1, s0, s1, f in jobs:
            out[b0:b1, s0:s1] = f.result()
    return out


def _shared_mlp(x, layers, n_global):
    for lyr in layers:
        x = x @ lyr["W"] + lyr["b"]
        mean = jax.lax.psum(jnp.sum(x, axis=(0, 1, 2)), "dp") / n_global
        var = jax.lax.psum(jnp.sum((x - mean) ** 2, axis=(0, 1, 2)), "dp") / n_global
        x = lyr["gamma"] * (x - mean) * jax.lax.rsqrt(var + EPS) + lyr["beta"]
        x = jax.nn.relu(x)
    return x


@partial(jax.pmap, axis_name="dp", in_axes=(0, 0, 0, None))
def _stage1(xyz, fps1, idx1, params):
    # xyz (b,N,3), fps1 (b,512) i32, idx1 (b,512,32) i32
    b = xyz.shape[0]
    bidx = jnp.arange(b)[:, None]
    b2 = jnp.arange(b)[:, None, None]
    new_xyz1 = xyz[bidx, fps1]                      # (b,512,3)
    g1 = xyz[b2, idx1] - new_xyz1[:, :, None]       # (b,512,32,3)
    f1 = jnp.max(_shared_mlp(g1, params["sa1"], float(B_FULL * 512 * 32)), axis=2)
    return new_xyz1, f1


@partial(jax.pmap, axis_name="dp", in_axes=(0, 0, 0, 0, None))
def _stage23(new_xyz1, f1, fps2, idx2, params):
    # new_xyz1 (b,512,3), f1 (b,512,128), fps2 (b,256) i32, idx2 (b,256,16) i32
    b = new_xyz1.shape[0]
    bidx = jnp.arange(b)[:, None]
    b2 = jnp.arange(b)[:, None, None]
    new_xyz2 = new_xyz1[bidx, fps2]                 # (b,256,3)
    g2 = jnp.concatenate(
        [new_xyz1[b2, idx2] - new_xyz2[:, :, None], f1[b2, idx2]], -1
    )                                               # (b,256,16,131)
    f2 = jnp.max(_shared_mlp(g2, params["sa2"], float(B_FULL * 256 * 16)), axis=2)
    g3 = jnp.concatenate([new_xyz2[:, None], f2[:, None]], -1)  # (b,1,256,259)
    f3 = jnp.max(_shared_mlp(g3, params["sa3"], float(B_FULL * 256)), axis=2)
    return jnp.transpose(f3, (0, 2, 1))             # (b,1024,1)


def _shard(a):
    return a.reshape((N_CORES, a.shape[0] // N_CORES) + a.shape[1:])


def kernel(pointcloud, params):
    xyz = np.ascontiguousarray(np.asarray(pointcloud, dtype=np.float32)[..., :3])
    params = jax.tree.map(lambda a: np.asarray(a, dtype=np.float32), params)
    B = xyz.shape[0]
    bidx = np.arange(B)[:, None]

    fps1 = _fps_np(xyz, 512)                        # (B,512) i32
    new_xyz1 = xyz[bidx, fps1]
    idx1 = _ball_query_np(xyz, new_xyz1, 0.04, 32)  # (B,512,32) i32

    # device: group + SA1 MLP; keep outputs device-resident
    d_new_xyz1, d_f1 = _stage1(_shard(xyz), _shard(fps1), _shard(idx1), params)

    # host: SA2 sampling on new_xyz1 (cheap, 512 pts)
    fps2 = _fps_np(new_xyz1, 256)                   # (B,256) i32
    new_xyz2 = new_xyz1[bidx, fps2]
    idx2 = _ball_query_np(new_xyz1, new_xyz2, 0.10, 16)  # (B,256,16) i32

    out = _stage23(d_new_xyz1, d_f1, _shard(fps2), _shard(idx2), params)
    return np.asarray(out, dtype=np.float32).reshape(B_FULL, 1024, 1)
